# revision 46
# baseline (speedup 1.0000x reference)
"""Multi-head attention (softmax over the QUERY axis) on 8 TRN2 NeuronCores.

Sharding: 2 batches x 4 head-groups (4 heads each) -> 8 cores.
Each core computes, for its (batch b, heads 4g..4g+3):
    qkT = W_{q,k} @ x_b^T + b_{q,k}   [512, 2048]   (e_out on partitions)
    V   = x_b @ W_v^T + b_v           [2048, 256]
    S'  = K Q^T (scores TRANSPOSED)   [k, q] per head
    P   = exp(S'/8) with fused row-sum -> denom[k]  (softmax over q == free dim)
    outT= sum_k (V[k,:]/denom[k]) P[k,:]            [d, q] per head
    part= outT^T @ WoT_g              [2048, 1024]  (partial for this head group)
Host sums the partials per batch (fp32) and adds bo.

Perf structure (measured 236.5us on HW, from a 256us baseline):
  - Two co-rooflines: ACT (the only exp engine; 128 exp calls of
    [128,1024] + fused accumulator reads ~ 160us busy) and the PE
    (~168us of streamed matmul columns at the warm 2.4 GHz clock; MM
    "pairing" via row/col groups does NOT add throughput -- concurrent
    MMs share the single rhs stream port).
  - A dense block of dummy PE matmuls at t=0 warms the HAM clock gate
    (PE 1.2 -> 2.4 GHz) while the input DMAs land; inputs are
    host-swizzled to contiguous per-partition blocks and split across
    the SP/ACT HWDGE rings + SWDGE ring so the qk chain unlocks early.
  - Per k-tile the emission order is [scores half0 | attn.V of the
    previous group | scores half1 | fillers/projections] so neither
    ACT nor the (sometimes cold-clocked) PE head-of-line blocks the
    exp stream.  Scores matmuls are emitted hh-adjacent.
  - qk/V filler groups run inside pair0's loop; pair1's loop instead
    carries its own K tiles, pair0's projection, and Q half1 --
    balancing PE load across both ACT-paced loops.
  - V is stored bf16 so the per-k V/denom scaling runs in DVE 4x mode;
    denominator reciprocals are batched per 2-ktile pair.
  - attn.V accumulates per 4-ktile group in PSUM; group flushes go
    straight into the SBUF fp32 accumulator, the last group's flush
    emits bf16 directly (no separate cast pass).
  - Projection outputs are written as bf16 partials (summed in fp32 on
    the host); the p1 tail alternates its PSUM->SBUF copies between
    DVE and the then-idle ACT, interleaves the half0 projection with
    half1's attn.V tail, and spreads output DMA over two rings.
"""

import sys

if "/opt/trn_rl_repo" not in sys.path:
    sys.path.insert(0, "/opt/trn_rl_repo")

import numpy as np
import ml_dtypes

import concourse.bass as bass
import concourse.mybir as mybir
import concourse.tile as tile
from concourse import bacc
from concourse.bass_utils import run_bass_kernel_spmd

F32 = mybir.dt.float32
F16 = mybir.dt.float16
BF16 = mybir.dt.bfloat16
AF = mybir.ActivationFunctionType

B, S, E, H = 2, 2048, 1024, 16
HL = 4  # heads per core
DH = 64
QK = 512  # q+k out dims per core (2*HL*DH)
V3 = 768  # q+k+v out dims per core
NCORES = 8

ET = E // 128  # 8 e-tiles
ST = S // 128  # 16 s-tiles
SC = S // 512  # 4 s/q chunks of 512
KT = ST  # 16 k-tiles
FG = 4  # k-tiles per attn.V accumulation group
NWARM = 16  # dummy matmuls to warm the PE clock gate

LAST_RESULTS = None


def build_kernel():
    nc = bacc.Bacc("TRN2", target_bir_lowering=False, debug=False, num_devices=NCORES)

    # x chunks pre-swizzled on the host to [partition, et, 512] so each DMA
    # reads contiguous per-partition blocks (descriptor-efficient)
    xcs = [
        nc.dram_tensor(f"xc{sc}", [128, ET, 512], BF16, kind="ExternalInput")
        for sc in range(SC)
    ]
    wc = nc.dram_tensor("wc", [128, ET, V3], BF16, kind="ExternalInput")
    # (wc is host-swizzled W^T: [partition, et, v])
    bq = nc.dram_tensor("bq", [128, 4], F32, kind="ExternalInput")
    bv = nc.dram_tensor("bv", [1, 256], BF16, kind="ExternalInput")
    woT = nc.dram_tensor("woT", [2 * 128, E], BF16, kind="ExternalInput")
    out0 = nc.dram_tensor("out0", [S, E], BF16, kind="ExternalOutput")
    out1 = nc.dram_tensor("out1", [S, E], BF16, kind="ExternalOutput")

    with tile.TileContext(nc) as tc:
        with (
            tc.tile_pool(name="persist", bufs=1) as persist,
            tc.tile_pool(name="smalls", bufs=3) as smalls,
            tc.tile_pool(name="expp", bufs=2 * FG) as expp,
            tc.tile_pool(name="vsp", bufs=5) as vsp,
            tc.tile_pool(name="fout", bufs=6) as foutp,
            tc.tile_pool(name="mm_ps", bufs=2, space="PSUM") as mm_ps,
            tc.tile_pool(name="sp_ps", bufs=2, space="PSUM") as sp_ps,
            tc.tile_pool(name="ot_ps", bufs=1, space="PSUM") as ot_ps,
        ):
            qk_sb = persist.tile([128, 4, S], BF16, tag="qk")
            v_sb = persist.tile([128, ST, 256], BF16, tag="v")
            outT_f32 = persist.tile([128, 2, S], F32, tag="outT")
            outT_bf = persist.tile([128, 2, S], BF16, tag="outT_bf")
            bq_sb = persist.tile([128, 4], F32, tag="bq")
            bv_sb = persist.tile([1, 256], BF16, tag="bv")
            ones_sb = persist.tile([1, 512], BF16, tag="ones")
            zeros_sb = persist.tile([1, 512], BF16, tag="zeros")
            den_sb = persist.tile([128, KT, 2, 2], F32, tag="den")
            xt_sb = persist.tile([128, ET, S], BF16, tag="xt")
            wt_sb = persist.tile([128, ET, V3], BF16, tag="wt")
            wo_sb = persist.tile([128, 2, E], BF16, tag="wo")

            # ---- PE warm-up: dense dummy matmuls while DMAs land ---------
            nc.vector.memset(ones_sb[:], 1.0)
            nc.vector.memset(zeros_sb[:], 0.0)
            for _ in range(NWARM):
                wp = mm_ps.tile([128, 512], F32, tag="mmps")
                nc.tensor.matmul(
                    wp[:, 0:256], ones_sb[0:1, 0:128], ones_sb[0:1, 0:256],
                    start=True, stop=True,
                )

            # input DMAs, all host-swizzled to contiguous per-partition
            # blocks: wc et-chunks first on BOTH HWDGE rings (they gate the
            # whole qk chain), then the four 1MB x chunks
            for et in range(ET):
                dma_eng = nc.sync if et % 2 == 0 else nc.scalar
                dma_eng.dma_start(wt_sb[:, et, :], wc[:, et, :])
            for sc in range(SC):
                dma_eng = nc.sync if sc % 2 == 0 else nc.scalar
                dma_eng.dma_start(
                    xt_sb[:, :, sc * 512 : (sc + 1) * 512], xcs[sc][:, :, :]
                )
            nc.gpsimd.dma_start(bq_sb[:], bq[:])
            nc.gpsimd.dma_start(bv_sb[:], bv[:])
            for p in range(2):
                nc.gpsimd.dma_start(wo_sb[:, p, :], woT[p * 128 : (p + 1) * 128, :])

            # ---- emitters for qkT / V accumulation groups ----------------
            def emit_qk_group(eo, sc):
                pt = mm_ps.tile([128, 512], F32, tag="mmps")
                for et in range(ET):
                    nc.tensor.matmul(
                        pt[:],
                        wt_sb[:, et, eo * 128 : (eo + 1) * 128],
                        xt_sb[:, et, sc * 512 : (sc + 1) * 512],
                        start=(et == 0),
                        stop=(et == ET - 1),
                    )
                nc.vector.tensor_scalar_add(
                    qk_sb[:, eo, sc * 512 : (sc + 1) * 512],
                    in0=pt[:],
                    scalar1=bq_sb[:, eo : eo + 1],
                )

            def emit_v_group(st):
                pt = mm_ps.tile([128, 512], F32, tag="mmps")
                for et in range(ET):
                    nc.tensor.matmul(
                        pt[:, :256],
                        xt_sb[:, et, st * 128 : (st + 1) * 128],
                        wt_sb[:, et, QK:V3],
                        start=(et == 0),
                        stop=False,
                    )
                nc.tensor.matmul(  # + ones^T bv (bias row)
                    pt[:, :256],
                    ones_sb[0:1, 0:128],
                    bv_sb[0:1, :],
                    start=False,
                    stop=True,
                )
                nc.vector.tensor_copy(v_sb[:, st, :], pt[:, :256])

            def emit_d_group(p, st, out_dram, tail=False, pads=0):
                # `pads` appends tiny N=64 no-op accumulations (+= 1^T @ 0)
                # to each projection matmul: ~110ns of PE busy-work apiece
                # that keeps the HAM clock gate warm without extra PSUM banks
                ot = foutp.tile([128, E], BF16, tag="fout", name=f"fo_{p}_{st}")
                for nck in range(2):
                    pt = mm_ps.tile([128, 512], F32, tag="mmps", name=f"fp_{p}_{st}_{nck}")
                    nc.tensor.matmul(
                        pt[:],
                        outT_bf[:, p, st * 128 : (st + 1) * 128],
                        wo_sb[:, p, nck * 512 : (nck + 1) * 512],
                        start=True,
                        stop=(pads == 0),
                    )
                    for i in range(pads):
                        nc.tensor.matmul(
                            pt[:, 0:64],
                            ones_sb[0:1, 0:128],
                            zeros_sb[0:1, 0:64],
                            start=False,
                            stop=(i == pads - 1),
                        )
                    if tail and nck == 1:
                        nc.scalar.copy(ot[:, nck * 512 : (nck + 1) * 512], pt[:])
                    else:
                        nc.vector.tensor_copy(ot[:, nck * 512 : (nck + 1) * 512], pt[:])
                dma_eng = nc.sync if st % 2 == 0 else nc.gpsimd
                dma_eng.dma_start(out_dram[st * 128 : (st + 1) * 128, :], ot[:])

            # ---- pre-attention: just enough for pair0 kt0 ----------------
            # Emission order IS program order: every filler must be emitted
            # no later than the k-tile iteration that first consumes it
            # (fillers pop at the TOP of each k-tile iteration).
            emit_qk_group(0, 0)  # Q heads 0,1 cols 0-511
            emit_qk_group(0, 1)
            emit_qk_group(2, 0)  # K heads 0,1 cols 0-511 (kts 0-3)

            def qg(eo, sc):
                return lambda: emit_qk_group(eo, sc)

            def vg(st):
                return lambda: emit_v_group(st)

            # p1's Q half1 / K chunks 1-3 are deferred into p1's own loop to
            # rebalance PE load (p0's loop is PE-bound, p1's has ACT slack)
            fillers = (
                [vg(0), vg(1), qg(2, 1), vg(2), vg(3), qg(2, 2), vg(4), qg(2, 3)]
                + [vg(5), vg(6), vg(7), vg(8)]
                + [qg(1, 0), qg(1, 1)]
                + [vg(9), vg(10)]
                + [qg(3, 0)]
                + [vg(11), vg(12), vg(13), vg(14), vg(15)]
            )
            fillers.reverse()  # pop() from the front

            # ---- attention per head pair ---------------------------------
            # attn.V slices for group g are spread over group g+1's k-tiles
            # (2 of a half's 4 j-steps per k-tile) so the PE load per k-tile
            # is even and the exp stream never sees a burst.
            c_state = {}

            def emit_c_slices(p, g, half, jpair, exs, vss):
                if jpair == 0:
                    c_state[half] = ot_ps.tile(
                        [128, 1024], F32, tag="otps", name=f"oTt_{p}_{g}_{half}"
                    )
                oTt = c_state[half]
                for j in (2 * jpair, 2 * jpair + 1):
                    kt = FG * g + j
                    vs_g, jj = vss[kt]
                    for qc in range(2):
                        for hh in range(2):  # hh-adjacent: disjoint col groups
                            q0 = half * 1024 + qc * 512
                            nc.tensor.matmul(
                                oTt[
                                    hh * 64 : (hh + 1) * 64,
                                    qc * 512 : (qc + 1) * 512,
                                ],
                                vs_g[:, jj, hh, :],
                                exs[kt][:, hh, q0 : q0 + 512],
                                start=(j == 0),
                                stop=(j == FG - 1),
                            )
                if jpair == 1:
                    f32dst = outT_f32[:, p, half * 1024 : (half + 1) * 1024]
                    if g == 0:
                        nc.vector.tensor_copy(f32dst, oTt[:])
                    elif g < KT // FG - 1:
                        nc.vector.tensor_add(f32dst, f32dst, oTt[:])
                    else:  # final group: emit bf16 directly
                        nc.vector.tensor_add(
                            outT_bf[:, p, half * 1024 : (half + 1) * 1024],
                            f32dst,
                            oTt[:],
                        )

            for p in range(2):
                exs = {}
                vss = {}

                def emit_scores_half(p, kt, half, ex):
                    # two fp32 PSUM tiles (one per head); matmuls interleaved
                    # hh-adjacent so consecutive MMs hit disjoint stationary
                    # row groups (PE row-tiling concurrency)
                    sps = [
                        sp_ps.tile([128, 1024], F32, tag="sp", name=f"sp{p}_{kt}_{half}_{hh}")
                        for hh in range(2)
                    ]
                    for qc in range(2):
                        for hh in range(2):
                            q0 = half * 1024 + qc * 512
                            nc.tensor.matmul(
                                sps[hh][:, qc * 512 : (qc + 1) * 512],
                                qk_sb[
                                    hh * 64 : (hh + 1) * 64,
                                    2 + p,
                                    kt * 128 : (kt + 1) * 128,
                                ],
                                qk_sb[hh * 64 : (hh + 1) * 64, p, q0 : q0 + 512],
                                start=True,
                                stop=True,
                            )
                    for hh in range(2):
                        nc.scalar.activation(
                            ex[:, hh, half * 1024 : (half + 1) * 1024],
                            sps[hh][:],
                            AF.Exp,
                            scale=0.125,
                            accum_out=den_sb[:, kt, hh, half : half + 1],
                        )

                for kt in range(KT):
                    ex = expp.tile([128, 2, S], BF16, tag="exp")
                    exs[kt] = ex
                    emit_scores_half(p, kt, 0, ex)
                    if kt == 0:
                        emit_qk_group(p, 2)  # Q cols 1024-2047 for half1
                        emit_qk_group(p, 3)
                    # previous group's attn.V between the two scores halves so
                    # the PE has queued work while ACT drains half0's exps
                    if kt >= FG:
                        o = kt % FG
                        emit_c_slices(p, kt // FG - 1, o // 2, o % 2, exs, vss)
                    emit_scores_half(p, kt, 1, ex)
                    # PE fillers (producers before their consumers)
                    if p == 0:
                        for _ in range(2):
                            if fillers:
                                fillers.pop()()
                    elif kt < 4:  # pair1 kt1-3: remaining K tiles for heads 2,3
                        if kt > 0:
                            emit_qk_group(3, kt)
                    else:  # pair1: overlap pair0's projection (padded to keep
                        # the PE dense enough that the clock gate stays warm)
                        emit_d_group(0, kt - 4, out0, pads=4)
                        if kt >= 12:
                            emit_d_group(0, kt, out0, pads=4)
                    # batched denominator bookkeeping per 2-ktile pair
                    if kt % 2 == 1:
                        k0 = kt - 1
                        dsum = smalls.tile([128, 2, 2], F32, tag="dsum")
                        nc.vector.tensor_add(
                            dsum[:],
                            den_sb[:, k0 : k0 + 2, :, 0],
                            den_sb[:, k0 : k0 + 2, :, 1],
                        )
                        rec = smalls.tile([128, 2, 2], F32, tag="rec")
                        nc.vector.reciprocal(rec[:], dsum[:])
                        vs_g = vsp.tile([128, 2, 2, DH], BF16, tag="vs")
                        for j in range(2):
                            vss[k0 + j] = (vs_g, j)
                            for hh in range(2):
                                nc.vector.tensor_scalar_mul(
                                    vs_g[:, j, hh, :],
                                    in0=v_sb[:, k0 + j, (2 * p + hh) * 64 : (2 * p + hh + 1) * 64],
                                    scalar1=rec[:, j, hh : hh + 1],
                                )
                # tail: last group's attn.V (both q-halves) + flush; for p1
                # interleave the half0 projection with half1's attn.V tail
                gl = KT // FG - 1
                emit_c_slices(p, gl, 0, 0, exs, vss)
                emit_c_slices(p, gl, 0, 1, exs, vss)
                if p == 0:
                    emit_c_slices(p, gl, 1, 0, exs, vss)
                    emit_c_slices(p, gl, 1, 1, exs, vss)
                else:
                    emit_d_group(1, 0, out1, tail=True, pads=4)
                    emit_d_group(1, 1, out1, tail=True, pads=4)
                    emit_c_slices(p, gl, 1, 0, exs, vss)
                    emit_d_group(1, 2, out1, tail=True, pads=4)
                    emit_d_group(1, 3, out1, tail=True, pads=4)
                    emit_c_slices(p, gl, 1, 1, exs, vss)
                    for st in range(4, ST):
                        emit_d_group(1, st, out1, tail=True, pads=4)


    nc.compile()
    return nc


def _shard_inputs(input, Wqkv, bqkv, Wo):
    """Build the 8 per-core input dicts (host-side layout/sharding)."""
    bf16 = ml_dtypes.bfloat16
    in_maps = []
    for c in range(NCORES):
        b = c // 4
        g = c % 4
        heads = range(4 * g, 4 * g + 4)
        rows = (
            [slice(64 * h, 64 * h + 64) for h in heads]
            + [slice(E + 64 * h, E + 64 * h + 64) for h in heads]
            + [slice(2 * E + 64 * h, 2 * E + 64 * h + 64) for h in heads]
        )
        W_sel = np.concatenate([Wqkv[s] for s in rows], axis=0)  # [768, 1024]
        b_sel = np.concatenate([bqkv[s] for s in rows], axis=0)  # [768]
        # x^T swizzled to [partition, et, s] per 512-column chunk so the
        # device DMA reads contiguous per-partition blocks
        xT_sw = (
            input[b].T.astype(bf16).reshape(8, 128, S).transpose(1, 0, 2)
        )  # [128, et, S]
        in_maps.append(
            {
                **{
                    f"xc{sc}": np.ascontiguousarray(
                        xT_sw[:, :, sc * 512 : (sc + 1) * 512]
                    )
                    for sc in range(SC)
                },
                "wc": np.ascontiguousarray(
                    W_sel.T.astype(bf16).reshape(8, 128, V3).transpose(1, 0, 2)
                ),
                "bq": np.ascontiguousarray(b_sel[:QK].reshape(4, 128).T),
                "bv": np.ascontiguousarray(b_sel[QK:V3].reshape(1, 256)).astype(bf16),
                "woT": np.ascontiguousarray(
                    Wo[:, 4 * g * DH : 4 * (g + 1) * DH].T
                ).astype(bf16),
            }
        )
    return in_maps


def kernel(input, Wqkv, bqkv, Wo, bo, _trace=False):
    global LAST_RESULTS
    input = np.asarray(input, dtype=np.float32)
    Wqkv = np.asarray(Wqkv, dtype=np.float32)
    bqkv = np.asarray(bqkv, dtype=np.float32)
    Wo = np.asarray(Wo, dtype=np.float32)
    bo = np.asarray(bo, dtype=np.float32)

    nc = build_kernel()
    in_maps = _shard_inputs(input, Wqkv, bqkv, Wo)
    kwargs = {}
    if _trace:
        kwargs = dict(trace=True, trace_cores=[0])
    res = run_bass_kernel_spmd(nc, in_maps, core_ids=list(range(NCORES)), **kwargs)
    LAST_RESULTS = res

    out = np.zeros((B, S, E), dtype=np.float32)
    for c in range(NCORES):
        out[c // 4] += res.results[c]["out0"].astype(np.float32)
        out[c // 4] += res.results[c]["out1"].astype(np.float32)
    out += bo
    return out


# revision 47
# speedup vs baseline: 1.0578x; 1.0578x over previous
"""Multi-head attention (softmax over the QUERY axis) on 8 TRN2 NeuronCores.

Sharding: 2 batches x 4 head-groups (4 heads each) -> 8 cores.
Each core computes, for its (batch b, heads 4g..4g+3):
    qkT = W_{q,k} @ x_b^T + b_{q,k}   [512, 2048]   (e_out on partitions)
    V   = x_b @ W_v^T + b_v           [2048, 256]
    S'  = K Q^T (scores TRANSPOSED)   [k, q] per head
    P   = exp(S'/8) with fused row-sum -> denom[k]  (softmax over q == free dim)
    outT= sum_k (V[k,:]/denom[k]) P[k,:]            [d, q] per head
    part= outT^T @ WoT_g              [2048, 1024]  (partial for this head group)
Host sums the partials per batch (fp32) and adds bo.

Perf structure (measured 236.5us on HW, from a 256us baseline):
  - Two co-rooflines: ACT (the only exp engine; 128 exp calls of
    [128,1024] + fused accumulator reads ~ 160us busy) and the PE
    (~168us of streamed matmul columns at the warm 2.4 GHz clock; MM
    "pairing" via row/col groups does NOT add throughput -- concurrent
    MMs share the single rhs stream port).
  - A dense block of dummy PE matmuls at t=0 warms the HAM clock gate
    (PE 1.2 -> 2.4 GHz) while the input DMAs land; inputs are
    host-swizzled to contiguous per-partition blocks and split across
    the SP/ACT HWDGE rings + SWDGE ring so the qk chain unlocks early.
  - Per k-tile the emission order is [scores half0 | attn.V of the
    previous group | scores half1 | fillers/projections] so neither
    ACT nor the (sometimes cold-clocked) PE head-of-line blocks the
    exp stream.  Scores matmuls are emitted hh-adjacent.
  - qk/V filler groups run inside pair0's loop; pair1's loop instead
    carries its own K tiles, pair0's projection, and Q half1 --
    balancing PE load across both ACT-paced loops.
  - V is stored bf16 so the per-k V/denom scaling runs in DVE 4x mode;
    denominator reciprocals are batched per 2-ktile pair.
  - attn.V accumulates per 4-ktile group in PSUM; group flushes go
    straight into the SBUF fp32 accumulator, the last group's flush
    emits bf16 directly (no separate cast pass).
  - Projection outputs are written as bf16 partials (summed in fp32 on
    the host); the p1 tail alternates its PSUM->SBUF copies between
    DVE and the then-idle ACT, interleaves the half0 projection with
    half1's attn.V tail, and spreads output DMA over two rings.
"""

import sys

if "/opt/trn_rl_repo" not in sys.path:
    sys.path.insert(0, "/opt/trn_rl_repo")

import numpy as np
import ml_dtypes

import concourse.bass as bass
import concourse.mybir as mybir
import concourse.tile as tile
from concourse import bacc
from concourse.bass_utils import run_bass_kernel_spmd

F32 = mybir.dt.float32
F16 = mybir.dt.float16
BF16 = mybir.dt.bfloat16
AF = mybir.ActivationFunctionType

B, S, E, H = 2, 2048, 1024, 16
HL = 4  # heads per core
DH = 64
QK = 512  # q+k out dims per core (2*HL*DH)
V3 = 768  # q+k+v out dims per core
NCORES = 8

ET = E // 128  # 8 e-tiles
ST = S // 128  # 16 s-tiles
SC = S // 512  # 4 s/q chunks of 512
KT = ST  # 16 k-tiles
FG = 4  # k-tiles per attn.V accumulation group
NWARM = 16  # dummy matmuls to warm the PE clock gate

LAST_RESULTS = None


def build_kernel():
    nc = bacc.Bacc("TRN2", target_bir_lowering=False, debug=False, num_devices=NCORES)

    # x chunks pre-swizzled on the host to [partition, et, 512] so each DMA
    # reads contiguous per-partition blocks (descriptor-efficient)
    xcs = [
        nc.dram_tensor(f"xc{sc}", [128, ET, 512], BF16, kind="ExternalInput")
        for sc in range(SC)
    ]
    wc = nc.dram_tensor("wc", [128, ET, V3], BF16, kind="ExternalInput")
    # (wc is host-swizzled W^T: [partition, et, v])
    bq = nc.dram_tensor("bq", [128, 4], F32, kind="ExternalInput")
    bv = nc.dram_tensor("bv", [1, 256], BF16, kind="ExternalInput")
    woT = nc.dram_tensor("woT", [2 * 128, E], BF16, kind="ExternalInput")
    out0 = nc.dram_tensor("out0", [S, E], BF16, kind="ExternalOutput")
    out1 = nc.dram_tensor("out1", [S, E], BF16, kind="ExternalOutput")

    with tile.TileContext(nc) as tc:
        with (
            tc.tile_pool(name="persist", bufs=1) as persist,
            tc.tile_pool(name="smalls", bufs=3) as smalls,
            tc.tile_pool(name="expp", bufs=2 * FG) as expp,
            tc.tile_pool(name="vsp", bufs=5) as vsp,
            tc.tile_pool(name="fout", bufs=6) as foutp,
            tc.tile_pool(name="mm_ps", bufs=2, space="PSUM") as mm_ps,
            tc.tile_pool(name="sp_ps", bufs=2, space="PSUM") as sp_ps,
            tc.tile_pool(name="ot_ps", bufs=1, space="PSUM") as ot_ps,
        ):
            qk_sb = persist.tile([128, 4, S], BF16, tag="qk")
            v_sb = persist.tile([128, ST, 256], BF16, tag="v")
            outT_f32 = persist.tile([128, 2, S], F32, tag="outT")
            outT_bf = persist.tile([128, 2, S], BF16, tag="outT_bf")
            bq_sb = persist.tile([128, 4], F32, tag="bq")
            bv_sb = persist.tile([1, 256], BF16, tag="bv")
            ones_sb = persist.tile([1, 512], BF16, tag="ones")
            zeros_sb = persist.tile([1, 512], BF16, tag="zeros")
            den_sb = persist.tile([128, KT, 2, 2], F32, tag="den")
            xt_sb = persist.tile([128, ET, S], BF16, tag="xt")
            wt_sb = persist.tile([128, ET, V3], BF16, tag="wt")
            wo_sb = persist.tile([128, 2, E], BF16, tag="wo")

            # ---- PE warm-up: dense dummy matmuls while DMAs land ---------
            nc.vector.memset(ones_sb[:], 1.0)
            nc.vector.memset(zeros_sb[:], 0.0)
            for _ in range(NWARM):
                wp = mm_ps.tile([128, 512], F32, tag="mmps")
                nc.tensor.matmul(
                    wp[:, 0:256], ones_sb[0:1, 0:128], ones_sb[0:1, 0:256],
                    start=True, stop=True,
                )

            # input DMAs, all host-swizzled to contiguous per-partition
            # blocks: wc et-chunks first on BOTH HWDGE rings (they gate the
            # whole qk chain), then the four 1MB x chunks
            for et in range(ET):
                dma_eng = nc.sync if et % 2 == 0 else nc.scalar
                dma_eng.dma_start(wt_sb[:, et, :], wc[:, et, :])
            for sc in range(SC):
                dma_eng = nc.sync if sc % 2 == 0 else nc.scalar
                dma_eng.dma_start(
                    xt_sb[:, :, sc * 512 : (sc + 1) * 512], xcs[sc][:, :, :]
                )
            nc.gpsimd.dma_start(bq_sb[:], bq[:])
            nc.gpsimd.dma_start(bv_sb[:], bv[:])
            for p in range(2):
                nc.gpsimd.dma_start(wo_sb[:, p, :], woT[p * 128 : (p + 1) * 128, :])

            # ---- emitters for qkT / V accumulation groups ----------------
            def emit_qk_group(eo, sc):
                pt = mm_ps.tile([128, 512], F32, tag="mmps")
                for et in range(ET):
                    nc.tensor.matmul(
                        pt[:],
                        wt_sb[:, et, eo * 128 : (eo + 1) * 128],
                        xt_sb[:, et, sc * 512 : (sc + 1) * 512],
                        start=(et == 0),
                        stop=(et == ET - 1),
                    )
                nc.vector.tensor_scalar_add(
                    qk_sb[:, eo, sc * 512 : (sc + 1) * 512],
                    in0=pt[:],
                    scalar1=bq_sb[:, eo : eo + 1],
                )

            def emit_v_group(st):
                pt = mm_ps.tile([128, 512], F32, tag="mmps")
                for et in range(ET):
                    nc.tensor.matmul(
                        pt[:, :256],
                        xt_sb[:, et, st * 128 : (st + 1) * 128],
                        wt_sb[:, et, QK:V3],
                        start=(et == 0),
                        stop=False,
                    )
                nc.tensor.matmul(  # + ones^T bv (bias row)
                    pt[:, :256],
                    ones_sb[0:1, 0:128],
                    bv_sb[0:1, :],
                    start=False,
                    stop=True,
                )
                nc.vector.tensor_copy(v_sb[:, st, :], pt[:, :256])

            def emit_d_group(p, st, out_dram, tail=False, pads=0):
                # `pads` appends tiny N=64 no-op accumulations (+= 1^T @ 0)
                # to each projection matmul: ~110ns of PE busy-work apiece
                # that keeps the HAM clock gate warm without extra PSUM banks
                ot = foutp.tile([128, E], BF16, tag="fout", name=f"fo_{p}_{st}")
                for nck in range(2):
                    pt = mm_ps.tile([128, 512], F32, tag="mmps", name=f"fp_{p}_{st}_{nck}")
                    nc.tensor.matmul(
                        pt[:],
                        outT_bf[:, p, st * 128 : (st + 1) * 128],
                        wo_sb[:, p, nck * 512 : (nck + 1) * 512],
                        start=True,
                        stop=(pads == 0),
                    )
                    for i in range(pads):
                        nc.tensor.matmul(
                            pt[:, 0:64],
                            ones_sb[0:1, 0:128],
                            zeros_sb[0:1, 0:64],
                            start=False,
                            stop=(i == pads - 1),
                        )
                    if tail and nck == 1:
                        nc.scalar.copy(ot[:, nck * 512 : (nck + 1) * 512], pt[:])
                    else:
                        nc.vector.tensor_copy(ot[:, nck * 512 : (nck + 1) * 512], pt[:])
                dma_eng = nc.sync if st % 2 == 0 else nc.gpsimd
                dma_eng.dma_start(out_dram[st * 128 : (st + 1) * 128, :], ot[:])

            # ---- pre-attention: just enough for pair0 kt0 ----------------
            # Emission order IS program order: every filler must be emitted
            # no later than the k-tile iteration that first consumes it
            # (fillers pop at the TOP of each k-tile iteration).
            emit_qk_group(0, 0)  # Q heads 0,1 cols 0-511
            emit_qk_group(0, 1)
            emit_qk_group(2, 0)  # K heads 0,1 cols 0-511 (kts 0-3)

            def qg(eo, sc):
                return lambda: emit_qk_group(eo, sc)

            def vg(st):
                return lambda: emit_v_group(st)

            # p1's Q half1 / K chunks 1-3 are deferred into p1's own loop to
            # rebalance PE load (p0's loop is PE-bound, p1's has ACT slack)
            fillers = (
                [vg(0), vg(1), qg(2, 1), vg(2), vg(3), qg(2, 2), vg(4), qg(2, 3)]
                + [vg(5), vg(6), vg(7), vg(8)]
                + [qg(1, 0), qg(1, 1)]
                + [vg(9), vg(10)]
                + [qg(3, 0)]
                + [vg(11), vg(12), vg(13), vg(14), vg(15)]
            )
            fillers.reverse()  # pop() from the front

            # ---- attention per head pair ---------------------------------
            # attn.V slices for group g are spread over group g+1's k-tiles
            # (2 of a half's 4 j-steps per k-tile) so the PE load per k-tile
            # is even and the exp stream never sees a burst.
            c_state = {}

            def emit_c_slices(p, g, half, jpair, exs, vss):
                if jpair == 0:
                    c_state[half] = ot_ps.tile(
                        [128, 1024], F32, tag="otps", name=f"oTt_{p}_{g}_{half}"
                    )
                oTt = c_state[half]
                for j in (2 * jpair, 2 * jpair + 1):
                    kt = FG * g + j
                    vs_g, jj = vss[kt]
                    for qc in range(2):
                        for hh in range(2):  # hh-adjacent: disjoint col groups
                            q0 = half * 1024 + qc * 512
                            nc.tensor.matmul(
                                oTt[
                                    hh * 64 : (hh + 1) * 64,
                                    qc * 512 : (qc + 1) * 512,
                                ],
                                vs_g[:, jj, hh, :],
                                exs[kt][:, hh, q0 : q0 + 512],
                                start=(j == 0),
                                stop=(j == FG - 1),
                            )
                if jpair == 1:
                    f32dst = outT_f32[:, p, half * 1024 : (half + 1) * 1024]
                    if g == 0:
                        nc.vector.tensor_copy(f32dst, oTt[:])
                    elif g < KT // FG - 1:
                        nc.vector.tensor_add(f32dst, f32dst, oTt[:])
                    else:  # final group: emit bf16 directly
                        nc.vector.tensor_add(
                            outT_bf[:, p, half * 1024 : (half + 1) * 1024],
                            f32dst,
                            oTt[:],
                        )

            for p in range(2):
                exs = {}
                vss = {}

                def emit_scores_half(p, kt, half, ex):
                    # two fp32 PSUM tiles (one per head); matmuls interleaved
                    # hh-adjacent so consecutive MMs hit disjoint stationary
                    # row groups (PE row-tiling concurrency)
                    sps = [
                        sp_ps.tile([128, 1024], F32, tag="sp", name=f"sp{p}_{kt}_{half}_{hh}")
                        for hh in range(2)
                    ]
                    for qc in range(2):
                        for hh in range(2):
                            q0 = half * 1024 + qc * 512
                            nc.tensor.matmul(
                                sps[hh][:, qc * 512 : (qc + 1) * 512],
                                qk_sb[
                                    hh * 64 : (hh + 1) * 64,
                                    2 + p,
                                    kt * 128 : (kt + 1) * 128,
                                ],
                                qk_sb[hh * 64 : (hh + 1) * 64, p, q0 : q0 + 512],
                                start=True,
                                stop=True,
                            )
                    for hh in range(2):
                        nc.scalar.activation(
                            ex[:, hh, half * 1024 : (half + 1) * 1024],
                            sps[hh][:],
                            AF.Exp,
                            scale=0.125,
                            accum_out=den_sb[:, kt, hh, half : half + 1],
                        )

                for kt in range(KT):
                    ex = expp.tile([128, 2, S], BF16, tag="exp")
                    exs[kt] = ex
                    emit_scores_half(p, kt, 0, ex)
                    if kt == 0:
                        emit_qk_group(p, 2)  # Q cols 1024-2047 for half1
                        emit_qk_group(p, 3)
                    # previous group's attn.V between the two scores halves so
                    # the PE has queued work while ACT drains half0's exps
                    if kt >= FG:
                        o = kt % FG
                        emit_c_slices(p, kt // FG - 1, o // 2, o % 2, exs, vss)
                    emit_scores_half(p, kt, 1, ex)
                    # PE fillers (producers before their consumers)
                    if p == 0:
                        for _ in range(2):
                            if fillers:
                                fillers.pop()()
                    elif kt < 4:  # pair1 kt1-3: remaining K tiles for heads 2,3
                        if kt > 0:
                            emit_qk_group(3, kt)
                    else:  # pair1: overlap pair0's projection (padded to keep
                        # the PE dense enough that the clock gate stays warm)
                        emit_d_group(0, kt - 4, out0)
                        if kt >= 12:
                            emit_d_group(0, kt, out0)
                    # batched denominator bookkeeping per 2-ktile pair
                    if kt % 2 == 1:
                        k0 = kt - 1
                        dsum = smalls.tile([128, 2, 2], F32, tag="dsum")
                        nc.vector.tensor_add(
                            dsum[:],
                            den_sb[:, k0 : k0 + 2, :, 0],
                            den_sb[:, k0 : k0 + 2, :, 1],
                        )
                        rec = smalls.tile([128, 2, 2], F32, tag="rec")
                        nc.vector.reciprocal(rec[:], dsum[:])
                        vs_g = vsp.tile([128, 2, 2, DH], BF16, tag="vs")
                        for j in range(2):
                            vss[k0 + j] = (vs_g, j)
                            for hh in range(2):
                                nc.vector.tensor_scalar_mul(
                                    vs_g[:, j, hh, :],
                                    in0=v_sb[:, k0 + j, (2 * p + hh) * 64 : (2 * p + hh + 1) * 64],
                                    scalar1=rec[:, j, hh : hh + 1],
                                )
                # tail: last group's attn.V (both q-halves) + flush; for p1
                # interleave the half0 projection with half1's attn.V tail
                gl = KT // FG - 1
                emit_c_slices(p, gl, 0, 0, exs, vss)
                emit_c_slices(p, gl, 0, 1, exs, vss)
                if p == 0:
                    emit_c_slices(p, gl, 1, 0, exs, vss)
                    emit_c_slices(p, gl, 1, 1, exs, vss)
                else:
                    emit_d_group(1, 0, out1, tail=True)
                    emit_d_group(1, 1, out1, tail=True)
                    emit_c_slices(p, gl, 1, 0, exs, vss)
                    emit_d_group(1, 2, out1, tail=True)
                    emit_d_group(1, 3, out1, tail=True)
                    emit_c_slices(p, gl, 1, 1, exs, vss)
                    for st in range(4, ST):
                        emit_d_group(1, st, out1, tail=True)


    nc.compile()
    return nc


def _shard_inputs(input, Wqkv, bqkv, Wo):
    """Build the 8 per-core input dicts (host-side layout/sharding)."""
    bf16 = ml_dtypes.bfloat16
    in_maps = []
    for c in range(NCORES):
        b = c // 4
        g = c % 4
        heads = range(4 * g, 4 * g + 4)
        rows = (
            [slice(64 * h, 64 * h + 64) for h in heads]
            + [slice(E + 64 * h, E + 64 * h + 64) for h in heads]
            + [slice(2 * E + 64 * h, 2 * E + 64 * h + 64) for h in heads]
        )
        W_sel = np.concatenate([Wqkv[s] for s in rows], axis=0)  # [768, 1024]
        b_sel = np.concatenate([bqkv[s] for s in rows], axis=0)  # [768]
        # x^T swizzled to [partition, et, s] per 512-column chunk so the
        # device DMA reads contiguous per-partition blocks
        xT_sw = (
            input[b].T.astype(bf16).reshape(8, 128, S).transpose(1, 0, 2)
        )  # [128, et, S]
        in_maps.append(
            {
                **{
                    f"xc{sc}": np.ascontiguousarray(
                        xT_sw[:, :, sc * 512 : (sc + 1) * 512]
                    )
                    for sc in range(SC)
                },
                "wc": np.ascontiguousarray(
                    W_sel.T.astype(bf16).reshape(8, 128, V3).transpose(1, 0, 2)
                ),
                "bq": np.ascontiguousarray(b_sel[:QK].reshape(4, 128).T),
                "bv": np.ascontiguousarray(b_sel[QK:V3].reshape(1, 256)).astype(bf16),
                "woT": np.ascontiguousarray(
                    Wo[:, 4 * g * DH : 4 * (g + 1) * DH].T
                ).astype(bf16),
            }
        )
    return in_maps


def kernel(input, Wqkv, bqkv, Wo, bo, _trace=False):
    global LAST_RESULTS
    input = np.asarray(input, dtype=np.float32)
    Wqkv = np.asarray(Wqkv, dtype=np.float32)
    bqkv = np.asarray(bqkv, dtype=np.float32)
    Wo = np.asarray(Wo, dtype=np.float32)
    bo = np.asarray(bo, dtype=np.float32)

    nc = build_kernel()
    in_maps = _shard_inputs(input, Wqkv, bqkv, Wo)
    kwargs = {}
    if _trace:
        kwargs = dict(trace=True, trace_cores=[0])
    res = run_bass_kernel_spmd(nc, in_maps, core_ids=list(range(NCORES)), **kwargs)
    LAST_RESULTS = res

    out = np.zeros((B, S, E), dtype=np.float32)
    for c in range(NCORES):
        out[c // 4] += res.results[c]["out0"].astype(np.float32)
        out[c // 4] += res.results[c]["out1"].astype(np.float32)
    out += bo
    return out


# revision 48
# speedup vs baseline: 1.0700x; 1.0116x over previous
"""Multi-head attention (softmax over the QUERY axis) on 8 TRN2 NeuronCores.

Sharding: 2 batches x 4 head-groups (4 heads each) -> 8 cores.
Each core computes, for its (batch b, heads 4g..4g+3):
    qkT = W_{q,k} @ x_b^T + b_{q,k}   [512, 2048]   (e_out on partitions)
    V   = x_b @ W_v^T + b_v           [2048, 256]
    S'  = K Q^T (scores TRANSPOSED)   [k, q] per head
    P   = exp(S'/8) with fused row-sum -> denom[k]  (softmax over q == free dim)
    outT= sum_k (V[k,:]/denom[k]) P[k,:]            [d, q] per head
    part= outT^T @ WoT_g              [2048, 1024]  (partial for this head group)
Host sums the partials per batch (fp32) and adds bo.

Perf structure (measured 236.5us on HW, from a 256us baseline):
  - Two co-rooflines: ACT (the only exp engine; 128 exp calls of
    [128,1024] + fused accumulator reads ~ 160us busy) and the PE
    (~168us of streamed matmul columns at the warm 2.4 GHz clock; MM
    "pairing" via row/col groups does NOT add throughput -- concurrent
    MMs share the single rhs stream port).
  - A dense block of dummy PE matmuls at t=0 warms the HAM clock gate
    (PE 1.2 -> 2.4 GHz) while the input DMAs land; inputs are
    host-swizzled to contiguous per-partition blocks and split across
    the SP/ACT HWDGE rings + SWDGE ring so the qk chain unlocks early.
  - Per k-tile the emission order is [scores half0 | attn.V of the
    previous group | scores half1 | fillers/projections] so neither
    ACT nor the (sometimes cold-clocked) PE head-of-line blocks the
    exp stream.  Scores matmuls are emitted hh-adjacent.
  - qk/V filler groups run inside pair0's loop; pair1's loop instead
    carries its own K tiles, pair0's projection, and Q half1 --
    balancing PE load across both ACT-paced loops.
  - V is stored bf16 so the per-k V/denom scaling runs in DVE 4x mode;
    denominator reciprocals are batched per 2-ktile pair.
  - attn.V accumulates per 4-ktile group in PSUM; group flushes go
    straight into the SBUF fp32 accumulator, the last group's flush
    emits bf16 directly (no separate cast pass).
  - Projection outputs are written as bf16 partials (summed in fp32 on
    the host); the p1 tail alternates its PSUM->SBUF copies between
    DVE and the then-idle ACT, interleaves the half0 projection with
    half1's attn.V tail, and spreads output DMA over two rings.
"""

import sys

if "/opt/trn_rl_repo" not in sys.path:
    sys.path.insert(0, "/opt/trn_rl_repo")

import numpy as np
import ml_dtypes

import concourse.bass as bass
import concourse.mybir as mybir
import concourse.tile as tile
from concourse import bacc
from concourse.bass_utils import run_bass_kernel_spmd

F32 = mybir.dt.float32
F16 = mybir.dt.float16
BF16 = mybir.dt.bfloat16
AF = mybir.ActivationFunctionType

B, S, E, H = 2, 2048, 1024, 16
HL = 4  # heads per core
DH = 64
QK = 512  # q+k out dims per core (2*HL*DH)
V3 = 768  # q+k+v out dims per core
NCORES = 8

ET = E // 128  # 8 e-tiles
ST = S // 128  # 16 s-tiles
SC = S // 512  # 4 s/q chunks of 512
KT = ST  # 16 k-tiles
FG = 4  # k-tiles per attn.V accumulation group
NWARM = 16  # dummy matmuls to warm the PE clock gate

LAST_RESULTS = None


def build_kernel():
    nc = bacc.Bacc("TRN2", target_bir_lowering=False, debug=False, num_devices=NCORES)

    # x chunks pre-swizzled on the host to [partition, et, 512] so each DMA
    # reads contiguous per-partition blocks (descriptor-efficient)
    xcs = [
        nc.dram_tensor(f"xc{sc}", [128, ET, 512], BF16, kind="ExternalInput")
        for sc in range(SC)
    ]
    wc = nc.dram_tensor("wc", [128, ET, V3], BF16, kind="ExternalInput")
    # (wc is host-swizzled W^T: [partition, et, v])
    bq = nc.dram_tensor("bq", [128, 4], F32, kind="ExternalInput")
    bv = nc.dram_tensor("bv", [1, 256], BF16, kind="ExternalInput")
    woT = nc.dram_tensor("woT", [2 * 128, E], BF16, kind="ExternalInput")
    out0 = nc.dram_tensor("out0", [S, E], BF16, kind="ExternalOutput")
    out1 = nc.dram_tensor("out1", [S, E], BF16, kind="ExternalOutput")

    with tile.TileContext(nc) as tc:
        with (
            tc.tile_pool(name="persist", bufs=1) as persist,
            tc.tile_pool(name="smalls", bufs=3) as smalls,
            tc.tile_pool(name="expp", bufs=2 * FG) as expp,
            tc.tile_pool(name="vsp", bufs=5) as vsp,
            tc.tile_pool(name="fout", bufs=6) as foutp,
            tc.tile_pool(name="mm_ps", bufs=2, space="PSUM") as mm_ps,
            tc.tile_pool(name="sp_ps", bufs=2, space="PSUM") as sp_ps,
            tc.tile_pool(name="ot_ps", bufs=1, space="PSUM") as ot_ps,
        ):
            qk_sb = persist.tile([128, 4, S], BF16, tag="qk")
            v_sb = persist.tile([128, ST, 256], BF16, tag="v")
            outT_f32 = persist.tile([128, 2, S], F32, tag="outT")
            outT_bf = persist.tile([128, 2, S], BF16, tag="outT_bf")
            bq_sb = persist.tile([128, 4], F32, tag="bq")
            bv_sb = persist.tile([1, 256], BF16, tag="bv")
            ones_sb = persist.tile([1, 512], BF16, tag="ones")
            zeros_sb = persist.tile([1, 512], BF16, tag="zeros")
            den_sb = persist.tile([128, KT, 2, 2], F32, tag="den")
            xt_sb = persist.tile([128, ET, S], BF16, tag="xt")
            wt_sb = persist.tile([128, ET, V3], BF16, tag="wt")
            wo_sb = persist.tile([128, 2, E], BF16, tag="wo")

            # ---- PE warm-up: dense dummy matmuls while DMAs land ---------
            nc.vector.memset(ones_sb[:], 1.0)
            nc.vector.memset(zeros_sb[:], 0.0)
            for _ in range(NWARM):
                wp = mm_ps.tile([128, 512], F32, tag="mmps")
                nc.tensor.matmul(
                    wp[:, 0:256], ones_sb[0:1, 0:128], ones_sb[0:1, 0:256],
                    start=True, stop=True,
                )

            # input DMAs, all host-swizzled to contiguous per-partition
            # blocks: wc et-chunks on the SWDGE ring; the four 1MB x chunks
            # split across the two HWDGE rings (SP + ACT)
            for et in range(ET):
                nc.gpsimd.dma_start(wt_sb[:, et, :], wc[:, et, :])
            for sc in range(SC):
                dma_eng = nc.sync if sc % 2 == 0 else nc.scalar
                dma_eng.dma_start(
                    xt_sb[:, :, sc * 512 : (sc + 1) * 512], xcs[sc][:, :, :]
                )
            nc.gpsimd.dma_start(bq_sb[:], bq[:])
            nc.gpsimd.dma_start(bv_sb[:], bv[:])
            for p in range(2):
                nc.gpsimd.dma_start(wo_sb[:, p, :], woT[p * 128 : (p + 1) * 128, :])

            # ---- emitters for qkT / V accumulation groups ----------------
            def emit_qk_group(eo, sc):
                pt = mm_ps.tile([128, 512], F32, tag="mmps")
                for et in range(ET):
                    nc.tensor.matmul(
                        pt[:],
                        wt_sb[:, et, eo * 128 : (eo + 1) * 128],
                        xt_sb[:, et, sc * 512 : (sc + 1) * 512],
                        start=(et == 0),
                        stop=(et == ET - 1),
                    )
                nc.vector.tensor_scalar_add(
                    qk_sb[:, eo, sc * 512 : (sc + 1) * 512],
                    in0=pt[:],
                    scalar1=bq_sb[:, eo : eo + 1],
                )

            def emit_v_group(st):
                pt = mm_ps.tile([128, 512], F32, tag="mmps")
                for et in range(ET):
                    nc.tensor.matmul(
                        pt[:, :256],
                        xt_sb[:, et, st * 128 : (st + 1) * 128],
                        wt_sb[:, et, QK:V3],
                        start=(et == 0),
                        stop=False,
                    )
                nc.tensor.matmul(  # + ones^T bv (bias row)
                    pt[:, :256],
                    ones_sb[0:1, 0:128],
                    bv_sb[0:1, :],
                    start=False,
                    stop=True,
                )
                nc.vector.tensor_copy(v_sb[:, st, :], pt[:, :256])

            def emit_d_group(p, st, out_dram, tail=False, pads=0):
                # `pads` appends tiny N=64 no-op accumulations (+= 1^T @ 0)
                # to each projection matmul: ~110ns of PE busy-work apiece
                # that keeps the HAM clock gate warm without extra PSUM banks
                ot = foutp.tile([128, E], BF16, tag="fout", name=f"fo_{p}_{st}")
                for nck in range(2):
                    pt = mm_ps.tile([128, 512], F32, tag="mmps", name=f"fp_{p}_{st}_{nck}")
                    nc.tensor.matmul(
                        pt[:],
                        outT_bf[:, p, st * 128 : (st + 1) * 128],
                        wo_sb[:, p, nck * 512 : (nck + 1) * 512],
                        start=True,
                        stop=(pads == 0),
                    )
                    for i in range(pads):
                        nc.tensor.matmul(
                            pt[:, 0:64],
                            ones_sb[0:1, 0:128],
                            zeros_sb[0:1, 0:64],
                            start=False,
                            stop=(i == pads - 1),
                        )
                    if tail and nck == 1:
                        nc.scalar.copy(ot[:, nck * 512 : (nck + 1) * 512], pt[:])
                    else:
                        nc.vector.tensor_copy(ot[:, nck * 512 : (nck + 1) * 512], pt[:])
                dma_eng = nc.sync if st % 2 == 0 else nc.gpsimd
                dma_eng.dma_start(out_dram[st * 128 : (st + 1) * 128, :], ot[:])

            # ---- pre-attention: just enough for pair0 kt0 ----------------
            # Emission order IS program order: every filler must be emitted
            # no later than the k-tile iteration that first consumes it
            # (fillers pop at the TOP of each k-tile iteration).
            emit_qk_group(0, 0)  # Q heads 0,1 cols 0-511
            emit_qk_group(0, 1)
            emit_qk_group(2, 0)  # K heads 0,1 cols 0-511 (kts 0-3)

            def qg(eo, sc):
                return lambda: emit_qk_group(eo, sc)

            def vg(st):
                return lambda: emit_v_group(st)

            # p1's Q half1 / K chunks 1-3 are deferred into p1's own loop to
            # rebalance PE load (p0's loop is PE-bound, p1's has ACT slack)
            fillers = (
                [vg(0), vg(1), qg(2, 1), vg(2), vg(3), qg(2, 2), vg(4), qg(2, 3)]
                + [vg(5), vg(6), vg(7), vg(8)]
                + [qg(1, 0), qg(1, 1)]
                + [vg(9), vg(10)]
                + [qg(3, 0)]
                + [vg(11), vg(12), vg(13), vg(14), vg(15)]
            )
            fillers.reverse()  # pop() from the front

            # ---- attention per head pair ---------------------------------
            # attn.V slices for group g are spread over group g+1's k-tiles
            # (2 of a half's 4 j-steps per k-tile) so the PE load per k-tile
            # is even and the exp stream never sees a burst.
            c_state = {}

            def emit_c_slices(p, g, half, jpair, exs, vss):
                if jpair == 0:
                    c_state[half] = ot_ps.tile(
                        [128, 1024], F32, tag="otps", name=f"oTt_{p}_{g}_{half}"
                    )
                oTt = c_state[half]
                for j in (2 * jpair, 2 * jpair + 1):
                    kt = FG * g + j
                    vs_g, jj = vss[kt]
                    for qc in range(2):
                        for hh in range(2):  # hh-adjacent: disjoint col groups
                            q0 = half * 1024 + qc * 512
                            nc.tensor.matmul(
                                oTt[
                                    hh * 64 : (hh + 1) * 64,
                                    qc * 512 : (qc + 1) * 512,
                                ],
                                vs_g[:, jj, hh, :],
                                exs[kt][:, hh, q0 : q0 + 512],
                                start=(j == 0),
                                stop=(j == FG - 1),
                            )
                if jpair == 1:
                    f32dst = outT_f32[:, p, half * 1024 : (half + 1) * 1024]
                    if g == 0:
                        nc.vector.tensor_copy(f32dst, oTt[:])
                    elif g < KT // FG - 1:
                        nc.vector.tensor_add(f32dst, f32dst, oTt[:])
                    else:  # final group: emit bf16 directly
                        nc.vector.tensor_add(
                            outT_bf[:, p, half * 1024 : (half + 1) * 1024],
                            f32dst,
                            oTt[:],
                        )

            for p in range(2):
                exs = {}
                vss = {}

                def emit_scores_half(p, kt, half, ex):
                    # two fp32 PSUM tiles (one per head); matmuls interleaved
                    # hh-adjacent so consecutive MMs hit disjoint stationary
                    # row groups (PE row-tiling concurrency)
                    sps = [
                        sp_ps.tile([128, 1024], F32, tag="sp", name=f"sp{p}_{kt}_{half}_{hh}")
                        for hh in range(2)
                    ]
                    for qc in range(2):
                        for hh in range(2):
                            q0 = half * 1024 + qc * 512
                            nc.tensor.matmul(
                                sps[hh][:, qc * 512 : (qc + 1) * 512],
                                qk_sb[
                                    hh * 64 : (hh + 1) * 64,
                                    2 + p,
                                    kt * 128 : (kt + 1) * 128,
                                ],
                                qk_sb[hh * 64 : (hh + 1) * 64, p, q0 : q0 + 512],
                                start=True,
                                stop=True,
                            )
                    for hh in range(2):
                        nc.scalar.activation(
                            ex[:, hh, half * 1024 : (half + 1) * 1024],
                            sps[hh][:],
                            AF.Exp,
                            scale=0.125,
                            accum_out=den_sb[:, kt, hh, half : half + 1],
                        )

                for kt in range(KT):
                    ex = expp.tile([128, 2, S], BF16, tag="exp")
                    exs[kt] = ex
                    emit_scores_half(p, kt, 0, ex)
                    if kt == 0:
                        emit_qk_group(p, 2)  # Q cols 1024-2047 for half1
                        emit_qk_group(p, 3)
                    # previous group's attn.V between the two scores halves so
                    # the PE has queued work while ACT drains half0's exps
                    if kt >= FG:
                        o = kt % FG
                        emit_c_slices(p, kt // FG - 1, o // 2, o % 2, exs, vss)
                    emit_scores_half(p, kt, 1, ex)
                    # PE fillers (producers before their consumers)
                    if p == 0:
                        for _ in range(2):
                            if fillers:
                                fillers.pop()()
                    elif kt < 4:  # pair1 kt1-3: remaining K tiles for heads 2,3
                        if kt > 0:
                            emit_qk_group(3, kt)
                    else:  # pair1: overlap pair0's projection (padded to keep
                        # the PE dense enough that the clock gate stays warm)
                        emit_d_group(0, kt - 4, out0)
                        if kt >= 12:
                            emit_d_group(0, kt, out0)
                    # batched denominator bookkeeping per 2-ktile pair
                    if kt % 2 == 1:
                        k0 = kt - 1
                        dsum = smalls.tile([128, 2, 2], F32, tag="dsum")
                        nc.vector.tensor_add(
                            dsum[:],
                            den_sb[:, k0 : k0 + 2, :, 0],
                            den_sb[:, k0 : k0 + 2, :, 1],
                        )
                        rec = smalls.tile([128, 2, 2], F32, tag="rec")
                        nc.vector.reciprocal(rec[:], dsum[:])
                        vs_g = vsp.tile([128, 2, 2, DH], BF16, tag="vs")
                        for j in range(2):
                            vss[k0 + j] = (vs_g, j)
                            for hh in range(2):
                                nc.vector.tensor_scalar_mul(
                                    vs_g[:, j, hh, :],
                                    in0=v_sb[:, k0 + j, (2 * p + hh) * 64 : (2 * p + hh + 1) * 64],
                                    scalar1=rec[:, j, hh : hh + 1],
                                )
                # tail: last group's attn.V (both q-halves) + flush; for p1
                # interleave the half0 projection with half1's attn.V tail
                gl = KT // FG - 1
                emit_c_slices(p, gl, 0, 0, exs, vss)
                emit_c_slices(p, gl, 0, 1, exs, vss)
                if p == 0:
                    emit_c_slices(p, gl, 1, 0, exs, vss)
                    emit_c_slices(p, gl, 1, 1, exs, vss)
                else:
                    emit_d_group(1, 0, out1, tail=True)
                    emit_d_group(1, 1, out1, tail=True)
                    emit_c_slices(p, gl, 1, 0, exs, vss)
                    emit_d_group(1, 2, out1, tail=True)
                    emit_d_group(1, 3, out1, tail=True)
                    emit_c_slices(p, gl, 1, 1, exs, vss)
                    for st in range(4, ST):
                        emit_d_group(1, st, out1, tail=True)


    nc.compile()
    return nc


def _shard_inputs(input, Wqkv, bqkv, Wo):
    """Build the 8 per-core input dicts (host-side layout/sharding)."""
    bf16 = ml_dtypes.bfloat16
    in_maps = []
    for c in range(NCORES):
        b = c // 4
        g = c % 4
        heads = range(4 * g, 4 * g + 4)
        rows = (
            [slice(64 * h, 64 * h + 64) for h in heads]
            + [slice(E + 64 * h, E + 64 * h + 64) for h in heads]
            + [slice(2 * E + 64 * h, 2 * E + 64 * h + 64) for h in heads]
        )
        W_sel = np.concatenate([Wqkv[s] for s in rows], axis=0)  # [768, 1024]
        b_sel = np.concatenate([bqkv[s] for s in rows], axis=0)  # [768]
        # x^T swizzled to [partition, et, s] per 512-column chunk so the
        # device DMA reads contiguous per-partition blocks
        xT_sw = (
            input[b].T.astype(bf16).reshape(8, 128, S).transpose(1, 0, 2)
        )  # [128, et, S]
        in_maps.append(
            {
                **{
                    f"xc{sc}": np.ascontiguousarray(
                        xT_sw[:, :, sc * 512 : (sc + 1) * 512]
                    )
                    for sc in range(SC)
                },
                "wc": np.ascontiguousarray(
                    W_sel.T.astype(bf16).reshape(8, 128, V3).transpose(1, 0, 2)
                ),
                "bq": np.ascontiguousarray(b_sel[:QK].reshape(4, 128).T),
                "bv": np.ascontiguousarray(b_sel[QK:V3].reshape(1, 256)).astype(bf16),
                "woT": np.ascontiguousarray(
                    Wo[:, 4 * g * DH : 4 * (g + 1) * DH].T
                ).astype(bf16),
            }
        )
    return in_maps


def kernel(input, Wqkv, bqkv, Wo, bo, _trace=False):
    global LAST_RESULTS
    input = np.asarray(input, dtype=np.float32)
    Wqkv = np.asarray(Wqkv, dtype=np.float32)
    bqkv = np.asarray(bqkv, dtype=np.float32)
    Wo = np.asarray(Wo, dtype=np.float32)
    bo = np.asarray(bo, dtype=np.float32)

    nc = build_kernel()
    in_maps = _shard_inputs(input, Wqkv, bqkv, Wo)
    kwargs = {}
    if _trace:
        kwargs = dict(trace=True, trace_cores=[0])
    res = run_bass_kernel_spmd(nc, in_maps, core_ids=list(range(NCORES)), **kwargs)
    LAST_RESULTS = res

    out = np.zeros((B, S, E), dtype=np.float32)
    for c in range(NCORES):
        out[c // 4] += res.results[c]["out0"].astype(np.float32)
        out[c // 4] += res.results[c]["out1"].astype(np.float32)
    out += bo
    return out
